# revision 19
# baseline (speedup 1.0000x reference)
"""Trainium2 Bass kernel for nn_LASLNNet (complex-valued 4D CNN).

Strategy (8 NeuronCores, SPMD single program):
  - core c handles (batch b = c//2, spatial half h = c%2) -> 4 x 2 split.
  - All complex convs are computed as real matmuls with doubled channels:
      [yr; yi] = [[Wr, Wi], [-Wi, Wr]]^T @ [xr; xi]
  - conv1 (k=3,s=2): im2col-lite slabs prepared on host (27 (j1,j2,j3) tap
    slabs; j4 handled as 3 PSUM-accumulated matmuls with step-2 rhs reads).
    Bias folded in via an all-ones K-channel so dummy edge rows stay zero.
    The slab DMA is split into R1 per-row chunks so conv1 row r starts as
    soon as chunk r lands; K and M are zero-padded to 128 so every matmul
    in the kernel runs in the same (128,128) PE tile mode.
  - conv2 (k=3,s=1,p=1): input stored on a d4-padded flat grid
    [block(d1) 7, d2 9, d3 9, d4 10] so each (j1,j2,j3) tap is a single
    flat offset; j4 in {0,1} fused into one K=128 matmul via a 1-element
    shifted replica of the input on partitions 64..127; j4=2 is a K=64
    matmul zero-padded to K=128. The replica is built with per-d1-block
    SBUF->SBUF DMAs that overlap conv1 compute (no bulk barrier).
    Edge taps restrict (o2,o3) ranges via strided APs; PSUM has_written
    semantics make partial-region accumulation correct (the first matmul
    of each group is the full-region interior tap). Within each PSUM
    group all 27 K=128 j4-fused taps are issued first, then the 27 j4=2
    taps, so the PE tile configuration never toggles mid-group.
  - conv3/4 (1x1): plain matmuls on a compact layout, interleaved per
    512-column chunk (conv3 both halves then conv4 both halves) so conv4
    starts while conv3 still runs on later chunks.
  - conv5 (1x1,s=2) then FC: on-chip mul+reduce against host-sliced fcw;
    final cross-half sum + fc bias on host (each core returns [128,1]).
  - dtype: bf16 matmul operands, fp32 PSUM/copies.

Spatial split along first output spatial dim D1 (9 rows):
  half 0 -> conv2..4 rows 0..4, half 1 -> rows 4..8 (row 4 duplicated);
  conv5 rows {0,1,2} / {2,3,4} (row 2 duplicated, masked via zeroed fcw).

_build_nc(reps=N) unrolls the steady-state body (x1 reload + all compute)
N times inside one NEFF; test.py uses (T(reps)-T(1))/(reps-1) to measure
the true per-iteration HW execution time independent of dispatch latency.
"""

import itertools

import numpy as np
import ml_dtypes

import concourse.bacc as bacc
import concourse.mybir as mybir
from concourse.tile import TileContext
from concourse.bass_utils import run_bass_kernel_spmd

F32 = mybir.dt.float32
BF16 = mybir.dt.bfloat16
BF = ml_dtypes.bfloat16

NB = 4            # batch
R1 = 7            # conv1 rows computed per core (incl. dummy edge rows)
R2 = 5            # conv2/3/4 rows per core
R5 = 3            # conv5 rows per core
D4P = 10          # d4-padded inner dim (9 valid + 1 zero)
D4S = 19          # raw d4 columns kept in the conv1 slab (col 19 never read)
BLK = 9 * 9 * D4P                # 810, one d1-block of x2
X2N = R1 * BLK                   # logical x2 elements per partition
CHK1 = 9 * 9 * D4S               # 1539, one conv1 slab row chunk
S1N = R1 * CHK1                  # conv1 slab elements per partition
# conv2 taps ordered interior-first so the first matmul of each PSUM group
# covers the full region (has_written correctness); weights are packed on
# the host in this same order so chunked weight DMAs land in consumption
# order.
TAPS = sorted(itertools.product(range(3), repeat=3),
              key=lambda t: (t != (1, 1, 1)))
N3 = R2 * 729                    # 3645 compact columns for conv3/4
N5 = R5 * 125                    # 375 conv5 output columns

_CACHE = {}


def _build_nc(reps=1):
    nc = bacc.Bacc("TRN2", target_bir_lowering=False, debug=False)

    x1_d = nc.dram_tensor("x1", [128, 4 * CHK1], BF16, kind="ExternalInput")
    w1_d = nc.dram_tensor("w1", [128, 6 * 128], BF16, kind="ExternalInput")
    w2a_d = nc.dram_tensor("w2a", [128, 27 * 128], BF16, kind="ExternalInput")
    w2b_d = nc.dram_tensor("w2b", [64, 27 * 128], BF16, kind="ExternalInput")
    bp_d = nc.dram_tensor("bp", [128, 6], F32, kind="ExternalInput")
    wp_d = nc.dram_tensor("wp", [128, 8 * 128], BF16, kind="ExternalInput")
    fcw_d = nc.dram_tensor("fcw", [128, N5], F32, kind="ExternalInput")
    out_d = nc.dram_tensor("out", [128, R5], F32, kind="ExternalOutput")

    Relu = mybir.ActivationFunctionType.Relu

    with TileContext(nc) as tc:
        with tc.tile_pool(name="sb", bufs=1) as pool, \
             tc.tile_pool(name="ps", bufs=3, space="PSUM") as pp, \
             tc.tile_pool(name="ps2", bufs=5, space="PSUM") as pp2:
            x1t = pool.tile([128, 4 * CHK1], BF16, tag="x1")
            w1t = pool.tile([128, 6 * 128], BF16, tag="w1")
            # x2 store: [1 lead margin][R1 blocks of BLK][91 tail margin]
            x2t = pool.tile([128, X2N + 92], BF16, tag="x2")
            w2at = pool.tile([128, 27 * 128], BF16, tag="w2a")
            w2bt = pool.tile([128, 27 * 128], BF16, tag="w2b")
            bpt = pool.tile([128, 6], F32, tag="bp")
            x3t = pool.tile([128, N3], BF16, tag="x3")
            wpt = pool.tile([128, 8 * 128], BF16, tag="wp")
            x4t = pool.tile([128, 2 * N3], BF16, tag="x4")
            x4bt = pool.tile([128, 2 * N3], BF16, tag="x4b")
            x5t = pool.tile([128, N5], F32, tag="x5")
            fcwt = pool.tile([128, N5], F32, tag="fcw")
            prodt = pool.tile([128, N5], F32, tag="prod")
            fct = pool.tile([128, R5], F32, tag="fc")
            b2t = bpt[:, 0:1]
            b3t = bpt[:, 1:3]
            b4t = bpt[:, 3:5]
            b5t = bpt[:, 5:6]
            w3t = wpt[:, 0:256]
            w4t = wpt[:, 256:768]
            w5t = wpt[:, 768:1024]

            # --- weight / bias loads (w1 first: conv1 needs it sooner) ---
            nc.sync.dma_start(w1t[:, :], w1_d[:, :])

            def load_x1(c, half):
                # chunk c carries conv1 rows c (parts 0-63) and c+4 (64-127);
                # halves split at the o2 4/5 boundary so each conv1 unit
                # starts as soon as its own half lands
                lo = c * CHK1 + (0 if half == 0 else 5 * 171)
                hi = c * CHK1 + (5 * 171 if half == 0 else CHK1)
                nc.sync.dma_start(x1t[:, lo:hi], x1_d[:, lo:hi])

            # x1 chunks first (conv2's first group needs conv1 rows 0..3),
            # then w2a in consumption-ordered thirds, then the rest.
            for c in range(4):
                load_x1(c, 0)
                load_x1(c, 1)
            for c in range(3):
                nc.sync.dma_start(w2at[:, c * 1152:(c + 1) * 1152],
                                  w2a_d[:, c * 1152:(c + 1) * 1152])
            nc.sync.dma_start(bpt[:, :], bp_d[:, :])

            # --- one-time zero fills ---
            nc.vector.memset(w2bt[64:128, :], 0)
            # x2 lead margin + tail margin (both partition halves)
            nc.vector.memset(x2t[:, 0:1], 0)
            nc.vector.memset(x2t[:, 1 + X2N:X2N + 92], 0)
            # d4 pad column of every (block, d2, d3) row
            x2pad = x2t[:, 1:1 + X2N].rearrange("p (x c) -> p x c", c=D4P)
            nc.vector.memset(x2pad[:, :, 9:10], 0)

            s1v = x1t.rearrange("p (r a b c) -> p r a b c", r=4, a=9, b=9, c=D4S)
            x2v = x2t[:, 1:1 + X2N].rearrange(
                "p (r a b c) -> p r a b c", r=R1, a=9, b=9, c=D4P)
            x3v = x3t.rearrange("p (r a b c) -> p r a b c", r=R2, a=9, b=9, c=9)
            x4bv = x4bt.rearrange("p (m r a b c) -> p m r a b c",
                                  m=2, r=R2, a=9, b=9, c=9)
            chunks = []
            pos = 0
            while pos < N3:
                sz = min(512, N3 - pos)
                chunks.append((pos, sz))
                pos += sz

            for rep in range(reps):
                if rep > 0:
                    # steady-state reload of the activations slab
                    for c in range(4):
                        load_x1(c, 0)
                        load_x1(c, 1)

                # ---------------- conv1 + shifted-replica build ----------------
                def conv1_unit(r):
                    v, rc = divmod(r, 4)
                    for gi, (o2s, c2g) in enumerate(((0, 5), (5, 4))):
                        n = c2g * 81
                        ps1 = pp.tile([128, 512], F32, tag="ps")
                        ps1v = ps1[:, :n].rearrange("p (a b c) -> p a b c",
                                                    a=c2g, b=9, c=9)
                        for j4 in range(3):
                            rhs = s1v[:, rc, o2s:o2s + c2g, :, j4:j4 + 17:2]
                            nc.tensor.matmul(
                                ps1v[:, :, :, :],
                                w1t[:, (v * 3 + j4) * 128:
                                       (v * 3 + j4 + 1) * 128],
                                rhs,
                                start=(j4 == 0), stop=(j4 == 2))
                        dst = x2v[0:64, r, o2s:o2s + c2g, :, 0:9]
                        src = ps1v[0:64, :, :, :]
                        # evacuate on alternating engines so the ACT chain
                        # doesn't delay the replica copies
                        if gi == 0:
                            nc.scalar.activation(dst, src, Relu)
                        else:
                            nc.vector.tensor_scalar(dst, src, 0.0, None,
                                                    mybir.AluOpType.max)

                def replica(r):
                    # shifted copy of block r (needs first elem of block r+1;
                    # the last block reads one col into the tail margin)
                    b0 = r * BLK
                    hi = b0 + BLK if r < R1 - 1 else X2N + 1
                    nc.sync.dma_start(x2t[64:128, b0:hi],
                                      x2t[0:64, b0 + 1:hi + 1])

                # row r lands with chunk r%4; conv2 runs its groups r=4..0 so
                # emit high rows/replicas first to match chunk arrival
                conv1_unit(4); conv1_unit(0)
                conv1_unit(5); conv1_unit(1)
                replica(4)
                conv1_unit(6); conv1_unit(2)
                replica(5); replica(6)
                conv1_unit(3)
                replica(3); replica(2); replica(1); replica(0)
                if rep == 0:
                    for c in range(3):
                        nc.sync.dma_start(w2bt[0:64, c * 1152:(c + 1) * 1152],
                                          w2b_d[:, c * 1152:(c + 1) * 1152])

                # ---------------- conv2 ----------------
                # taps restricted to the valid (o2, o3) window; returns the
                # PSUM out view and the x2 flat base of the window start
                def tap_geom(ps2v, r, o2s, c2g, j1, j2, j3):
                    blk = r + j1
                    lo2 = max(o2s, 1 - j2)
                    hi2 = min(o2s + c2g, 10 - j2)
                    lo3 = max(0, 1 - j3)
                    hi3 = min(9, 10 - j3)
                    c2 = hi2 - lo2
                    c3 = hi3 - lo3
                    out_ap = ps2v[:, lo2 - o2s:hi2 - o2s, lo3:hi3, :]
                    base0 = (blk * BLK + (lo2 + j2 - 1) * 90
                             + (lo3 + j3 - 1) * D4P)
                    return out_ap, base0, c2, c3

                def rhs_win(plo, phi, base, c2, c3):
                    return x2t[plo:phi, base:base + c2 * 90].rearrange(
                        "p (a b c) -> p a b c", a=c2, b=9, c=D4P)[
                        :, :, 0:c3, 0:9]

                for r in range(R2 - 1, -1, -1):
                    # both o2s-subgroups batched per pass so the PE tile mode
                    # toggles once per direction, not per tap
                    groups = []
                    for (o2s, c2g) in ((0, 5), (5, 4)):
                        n = c2g * 81
                        ps2 = pp.tile([128, 512], F32, tag="ps")
                        groups.append((ps2[:, :n].rearrange(
                            "p (a b c) -> p a b c", a=c2g, b=9, c=9),
                            o2s, c2g))
                    # pass 1: 27 j4-fused K=128 taps (interior first)
                    for (ps2v, o2s, c2g) in groups:
                        for ti, (j1, j2, j3) in enumerate(TAPS):
                            out_ap, base0, c2, c3 = tap_geom(
                                ps2v, r, o2s, c2g, j1, j2, j3)
                            nc.tensor.matmul(
                                out_ap,
                                w2at[:, ti * 128:(ti + 1) * 128],
                                rhs_win(0, 128, base0, c2, c3),
                                start=(ti == 0), stop=False)
                    # pass 2: 27 j4=2 taps, K zero-padded to 128 (the
                    # upper partitions read finite replica data times zero
                    # weights), keeping the PE in (128,128) tile mode
                    for (ps2v, o2s, c2g) in groups:
                        for ti, (j1, j2, j3) in enumerate(TAPS):
                            out_ap, base0, c2, c3 = tap_geom(
                                ps2v, r, o2s, c2g, j1, j2, j3)
                            nc.tensor.matmul(
                                out_ap,
                                w2bt[:, ti * 128:(ti + 1) * 128],
                                rhs_win(0, 128, base0 + 2, c2, c3),
                                start=False, stop=(ti == 26))
                    for gi, (ps2v, o2s, c2g) in enumerate(groups):
                        dst = x3v[:, r, o2s:o2s + c2g, :, :]
                        if gi == 0:
                            nc.scalar.activation(dst, ps2v[:, :, :, :],
                                                 Relu, bias=b2t[:, :])
                        else:
                            nc.vector.tensor_scalar(
                                dst, ps2v[:, :, :, :], b2t[:, :], 0.0,
                                mybir.AluOpType.add, mybir.AluOpType.max)

                if rep == 0:
                    # late weights: emitted after conv2 so the x2 replica
                    # copies outrank them on the DMA queue; they still land
                    # long before conv3 needs them.
                    nc.sync.dma_start(wpt[:, :], wp_d[:, :])
                    nc.sync.dma_start(fcwt[:, :], fcw_d[:, :])

                # ---------------- conv3 + conv4 (1x1), chunk-interleaved ----
                Amax = mybir.AluOpType.max
                Aadd = mybir.AluOpType.add

                def evac(dst, src, bias, mh):
                    # PSUM evacuation alternates engines: ScalarE handles
                    # mh=0, VectorE mh=1, so neither engine serializes PE.
                    if mh == 0:
                        nc.scalar.activation(dst, src, Relu, bias=bias)
                    else:
                        nc.vector.tensor_scalar(dst, src, bias, 0.0,
                                                Aadd, Amax)

                def conv3_chunk(pos, sz):
                    for mh in range(2):
                        ps3 = pp2.tile([128, 512], F32, tag="ps2")
                        nc.tensor.matmul(
                            ps3[:, :sz],
                            w3t[:, mh * 128:(mh + 1) * 128],
                            x3t[:, pos:pos + sz],
                            start=True, stop=True)
                        evac(x4t[:, mh * N3 + pos:mh * N3 + pos + sz],
                             ps3[:, :sz], b3t[:, mh:mh + 1], mh)

                def conv4_chunk(pos, sz):
                    for mh in range(2):
                        ps4 = pp2.tile([128, 512], F32, tag="ps2")
                        nc.tensor.matmul(
                            ps4[:, :sz],
                            w4t[:, (mh * 2) * 128:(mh * 2 + 1) * 128],
                            x4t[:, pos:pos + sz],
                            start=True, stop=False)
                        nc.tensor.matmul(
                            ps4[:, :sz],
                            w4t[:, (mh * 2 + 1) * 128:(mh * 2 + 2) * 128],
                            x4t[:, N3 + pos:N3 + pos + sz],
                            start=False, stop=True)
                        evac(x4bt[:, mh * N3 + pos:mh * N3 + pos + sz],
                             ps4[:, :sz], b4t[:, mh:mh + 1], mh)

                # two-chunk software pipeline: conv4(k) trails conv3(k+2)
                rchunks = list(reversed(chunks))
                conv3_chunk(*rchunks[0])
                conv3_chunk(*rchunks[1])
                for i in range(len(rchunks)):
                    if i + 2 < len(rchunks):
                        conv3_chunk(*rchunks[i + 2])
                    conv4_chunk(*rchunks[i])

                # ---------------- conv5 (1x1, s=2, 128c->64c) ----------------
                for rr in range(R5 - 1, -1, -1):
                    ps5 = pp2.tile([128, 512], F32, tag="ps2")
                    for mb in range(2):
                        rhs = x4bv[:, mb, 2 * rr, 0:9:2, 0:9:2, 0:9:2]
                        nc.tensor.matmul(
                            ps5[:, :125],
                            w5t[:, mb * 128:(mb + 1) * 128],
                            rhs,
                            start=(mb == 0), stop=(mb == 1))
                    nc.scalar.activation(
                        x5t[:, rr * 125:(rr + 1) * 125],
                        ps5[:, :125],
                        Relu, bias=b5t[:, :])
                    nc.vector.tensor_mul(
                        prodt[:, rr * 125:(rr + 1) * 125],
                        x5t[:, rr * 125:(rr + 1) * 125],
                        fcwt[:, rr * 125:(rr + 1) * 125])
                    nc.vector.reduce_sum(
                        fct[:, rr:rr + 1],
                        prodt[:, rr * 125:(rr + 1) * 125],
                        axis=mybir.AxisListType.X)


            nc.sync.dma_start(out_d[:, :], fct[:, :])

    nc.compile()
    return nc


# ---------------- host-side data prep ----------------

def _prep_weights(inputs):
    f32 = np.float32
    w1r = np.asarray(inputs["w1r"], f32)[:, 0]   # [32, 3,3,3,3]
    w1i = np.asarray(inputs["w1i"], f32)[:, 0]
    # [t27, j4, co]
    w1r_t = w1r.transpose(1, 2, 3, 4, 0).reshape(27, 3, 32)
    w1i_t = w1i.transpose(1, 2, 3, 4, 0).reshape(27, 3, 32)
    # two variants: cols [0:384) contract slab rows 0-3 (K rows 0-54),
    # cols [384:768) contract slab rows 4-6 (K rows 64-118)
    W1 = np.zeros((128, 6 * 128), f32)
    for v in range(2):
        k0 = 64 * v
        for j4 in range(3):
            c0 = (v * 3 + j4) * 128
            W1[k0 + 0:k0 + 27, c0:c0 + 32] = w1r_t[:, j4]
            W1[k0 + 0:k0 + 27, c0 + 32:c0 + 64] = w1i_t[:, j4]
            W1[k0 + 27:k0 + 54, c0:c0 + 32] = -w1i_t[:, j4]
            W1[k0 + 27:k0 + 54, c0 + 32:c0 + 64] = w1r_t[:, j4]
        W1[k0 + 54, v * 3 * 128:v * 3 * 128 + 32] = np.asarray(inputs["b1r"], f32)
        W1[k0 + 54, v * 3 * 128 + 32:v * 3 * 128 + 64] = np.asarray(inputs["b1i"], f32)

    w2r = np.asarray(inputs["w2r"], f32)   # [64, 32, 3,3,3,3]
    w2i = np.asarray(inputs["w2i"], f32)
    # [t27, j4, ci, co]
    w2r_t = w2r.transpose(2, 3, 4, 5, 1, 0).reshape(27, 3, 32, 64)
    w2i_t = w2i.transpose(2, 3, 4, 5, 1, 0).reshape(27, 3, 32, 64)
    W2a = np.zeros((128, 27 * 128), f32)
    W2b = np.zeros((64, 27 * 128), f32)
    # columns packed in TAPS (kernel emission) order
    for ti, (j1, j2, j3) in enumerate(TAPS):
        t = j1 * 9 + j2 * 3 + j3
        for jj, r0 in ((0, 0), (1, 64)):
            W2a[r0 + 0:r0 + 32, ti * 128:ti * 128 + 64] = w2r_t[t, jj]
            W2a[r0 + 0:r0 + 32, ti * 128 + 64:(ti + 1) * 128] = w2i_t[t, jj]
            W2a[r0 + 32:r0 + 64, ti * 128:ti * 128 + 64] = -w2i_t[t, jj]
            W2a[r0 + 32:r0 + 64, ti * 128 + 64:(ti + 1) * 128] = w2r_t[t, jj]
        W2b[0:32, ti * 128:ti * 128 + 64] = w2r_t[t, 2]
        W2b[0:32, ti * 128 + 64:(ti + 1) * 128] = w2i_t[t, 2]
        W2b[32:64, ti * 128:ti * 128 + 64] = -w2i_t[t, 2]
        W2b[32:64, ti * 128 + 64:(ti + 1) * 128] = w2r_t[t, 2]
    B2 = np.concatenate([np.asarray(inputs["b2r"], f32),
                         np.asarray(inputs["b2i"], f32)])[:, None]

    w3r = np.asarray(inputs["w3r"], f32).reshape(128, 64)
    w3i = np.asarray(inputs["w3i"], f32).reshape(128, 64)
    W3 = np.zeros((128, 2 * 128), f32)
    W3[0:64, 0:128] = w3r.T
    W3[64:128, 0:128] = -w3i.T
    W3[0:64, 128:256] = w3i.T
    W3[64:128, 128:256] = w3r.T
    B3 = np.stack([np.asarray(inputs["b3r"], f32),
                   np.asarray(inputs["b3i"], f32)], axis=1)

    w4r = np.asarray(inputs["w4r"], f32).reshape(128, 128)
    w4i = np.asarray(inputs["w4i"], f32).reshape(128, 128)
    W4 = np.zeros((128, 4 * 128), f32)
    W4[:, 0:128] = w4r.T
    W4[:, 128:256] = -w4i.T
    W4[:, 256:384] = w4i.T
    W4[:, 384:512] = w4r.T
    B4 = np.stack([np.asarray(inputs["b4r"], f32),
                   np.asarray(inputs["b4i"], f32)], axis=1)

    w5r = np.asarray(inputs["w5r"], f32).reshape(64, 128)
    w5i = np.asarray(inputs["w5i"], f32).reshape(64, 128)
    W5 = np.zeros((128, 2 * 128), f32)
    W5[:, 0:64] = w5r.T
    W5[:, 64:128] = w5i.T
    W5[:, 128:192] = -w5i.T
    W5[:, 192:256] = w5r.T
    B5 = np.concatenate([np.asarray(inputs["b5r"], f32),
                         np.asarray(inputs["b5i"], f32)])[:, None]

    BP = np.zeros((128, 6), f32)
    BP[:, 0:1] = B2
    BP[:, 1:3] = B3
    BP[:, 3:5] = B4
    BP[:, 5:6] = B5
    WP = np.zeros((128, 8 * 128), f32)
    WP[:, 0:256] = W3
    WP[:, 256:768] = W4
    WP[:, 768:1024] = W5
    return {
        "w1": W1.astype(BF), "w2a": W2a.astype(BF), "w2b": W2b.astype(BF),
        "bp": BP, "wp": WP.astype(BF),
    }


def _prep_x1(xr_b, xi_b, h):
    """Conv1 input slab for one (batch, half), packed [128, 4*CHK1] bf16:
    partition p < 64 holds slab row r=chunk of tap p; partition 64+p holds
    row chunk+4 (row 7 slot is zero)."""
    S = np.zeros((64, R1 + 1, 9, 9, D4S), np.float32)
    glo = max(0, 4 * h - 1)
    ghi = min(8, 4 * h + 5)
    rlo = glo - (4 * h - 1)
    rhi = ghi - (4 * h - 1) + 1
    for t, (j1, j2, j3) in enumerate(itertools.product(range(3), repeat=3)):
        subr = xr_b[j1:j1 + 17:2, j2:j2 + 17:2, j3:j3 + 17:2, :D4S]
        subi = xi_b[j1:j1 + 17:2, j2:j2 + 17:2, j3:j3 + 17:2, :D4S]
        S[t, rlo:rhi] = subr[glo:ghi + 1]
        S[27 + t, rlo:rhi] = subi[glo:ghi + 1]
    S[54, rlo:rhi] = 1.0
    S = S.reshape(64, 2, 4 * CHK1)
    return np.concatenate([S[:, 0], S[:, 1]], axis=0).astype(BF)


def _prep_fcw(fcw, h):
    out = np.zeros((128, N5), np.float32)
    f = np.asarray(fcw, np.float32).reshape(-1)
    for rr in range(R5):
        g5 = rr + 2 * h
        if h == 1 and rr == 0:
            continue  # overlap row masked on half 1
        out[:, rr * 125:(rr + 1) * 125] = f[g5 * 125:(g5 + 1) * 125][None, :]
    return out


def _make_in_maps(inputs):
    wkey = id(inputs.get("w1r"))
    if _CACHE.get("wkey") != wkey:
        _CACHE["wmaps"] = _prep_weights(inputs)
        _CACHE["wkey"] = wkey
    wmaps = _CACHE["wmaps"]
    xr = np.asarray(inputs["xr"], np.float32)
    xi = np.asarray(inputs["xi"], np.float32)
    fcw = inputs["fcw"]
    in_maps = []
    for core in range(8):
        b, h = core // 2, core % 2
        m = dict(wmaps)
        m["x1"] = _prep_x1(xr[b, 0], xi[b, 0], h)
        m["fcw"] = _prep_fcw(fcw, h)
        in_maps.append(m)
    return in_maps


def kernel(**inputs):
    if "nc" not in _CACHE:
        _CACHE["nc"] = _build_nc()
    nc = _CACHE["nc"]

    in_maps = _make_in_maps(inputs)
    res = run_bass_kernel_spmd(nc, in_maps, core_ids=list(range(8)))

    fcb = np.asarray(inputs["fcb"], np.float32)
    yr = np.zeros((NB, 64, 1), np.float32)
    yi = np.zeros((NB, 64, 1), np.float32)
    for b in range(NB):
        p0 = res.results[2 * b]["out"]
        p1 = res.results[2 * b + 1]["out"]
        s = (p0 + p1).sum(axis=1, keepdims=True)
        yr[b] = s[0:64] + fcb[0]
        yi[b] = s[64:128]
    return np.stack([yr, yi]).astype(np.float32)


# revision 21
# speedup vs baseline: 1.1179x; 1.1179x over previous
"""Trainium2 Bass kernel for nn_LASLNNet (complex-valued 4D CNN).

Strategy (8 NeuronCores, SPMD single program):
  - core c handles (batch b = c//2, spatial half h = c%2) -> 4 x 2 split.
  - All complex convs are computed as real matmuls with doubled channels:
      [yr; yi] = [[Wr, Wi], [-Wi, Wr]]^T @ [xr; xi]
  - conv1 (k=3,s=2): im2col-lite slabs prepared on host (27 (j1,j2,j3) tap
    slabs; j4 handled as 3 PSUM-accumulated matmuls with step-2 rhs reads).
    Bias folded in via an all-ones K-channel so dummy edge rows stay zero.
    The slab DMA is split into R1 per-row chunks so conv1 row r starts as
    soon as chunk r lands; K and M are zero-padded to 128 so every matmul
    in the kernel runs in the same (128,128) PE tile mode.
  - conv2 (k=3,s=1,p=1): input stored on a d4-padded flat grid
    [block(d1) 7, d2 9, d3 9, d4 10] so each (j1,j2,j3) tap is a single
    flat offset; j4 in {0,1} fused into one K=128 matmul via a 1-element
    shifted replica of the input on partitions 64..127; j4=2 is a K=64
    matmul zero-padded to K=128. The replica is built with per-d1-block
    SBUF->SBUF DMAs that overlap conv1 compute (no bulk barrier).
    Edge taps restrict (o2,o3) ranges via strided APs; PSUM has_written
    semantics make partial-region accumulation correct (the first matmul
    of each group is the full-region interior tap). Within each PSUM
    group all 27 K=128 j4-fused taps are issued first, then the 27 j4=2
    taps, so the PE tile configuration never toggles mid-group.
  - conv3/4 (1x1): plain matmuls on a compact layout, interleaved per
    512-column chunk (conv3 both halves then conv4 both halves) so conv4
    starts while conv3 still runs on later chunks.
  - conv5 (1x1,s=2) then FC: on-chip mul+reduce against host-sliced fcw;
    final cross-half sum + fc bias on host (each core returns [128,1]).
  - dtype: bf16 matmul operands, fp32 PSUM/copies.

Spatial split along first output spatial dim D1 (9 rows):
  half 0 -> conv2..4 rows 0..4, half 1 -> rows 4..8 (row 4 duplicated);
  conv5 rows {0,1,2} / {2,3,4} (row 2 duplicated, masked via zeroed fcw).

_build_nc(reps=N) unrolls the steady-state body (x1 reload + all compute)
N times inside one NEFF; test.py uses (T(reps)-T(1))/(reps-1) to measure
the true per-iteration HW execution time independent of dispatch latency.
"""

import itertools

import numpy as np
import ml_dtypes

import concourse.bacc as bacc
import concourse.mybir as mybir
from concourse.tile import TileContext
from concourse.bass_utils import run_bass_kernel_spmd

F32 = mybir.dt.float32
BF16 = mybir.dt.bfloat16
BF = ml_dtypes.bfloat16

NB = 4            # batch
R1 = 7            # conv1 rows computed per core (incl. dummy edge rows)
R2 = 5            # conv2/3/4 rows per core
R5 = 3            # conv5 rows per core
D4P = 10          # d4-padded inner dim (9 valid + 1 zero)
D4S = 19          # raw d4 columns kept in the conv1 slab (col 19 never read)
BLK = 9 * 9 * D4P                # 810, one d1-block of x2
X2N = R1 * BLK                   # logical x2 elements per partition
CHK1 = 9 * 9 * D4S               # 1539, one conv1 slab row chunk
S1N = R1 * CHK1                  # conv1 slab elements per partition
# conv2 taps ordered interior-first so the first matmul of each PSUM group
# covers the full region (has_written correctness); weights are packed on
# the host in this same order so chunked weight DMAs land in consumption
# order.
TAPS = sorted(itertools.product(range(3), repeat=3),
              key=lambda t: (t != (1, 1, 1)))
TI = {t: i for i, t in enumerate(TAPS)}
N3 = R2 * 729                    # 3645 compact columns for conv3/4
N5 = R5 * 125                    # 375 conv5 output columns

_CACHE = {}


def _build_nc(reps=1):
    nc = bacc.Bacc("TRN2", target_bir_lowering=False, debug=False)

    x1_d = nc.dram_tensor("x1", [128, 4 * CHK1], BF16, kind="ExternalInput")
    w1_d = nc.dram_tensor("w1", [128, 6 * 128], BF16, kind="ExternalInput")
    w2a_d = nc.dram_tensor("w2a", [128, 27 * 128], BF16, kind="ExternalInput")
    w2b_d = nc.dram_tensor("w2b", [64, 27 * 128], BF16, kind="ExternalInput")
    w2p_d = nc.dram_tensor("w2p", [128, 9 * 128], BF16, kind="ExternalInput")
    bp_d = nc.dram_tensor("bp", [128, 6], F32, kind="ExternalInput")
    wp_d = nc.dram_tensor("wp", [128, 8 * 128], BF16, kind="ExternalInput")
    fcw_d = nc.dram_tensor("fcw", [128, N5], F32, kind="ExternalInput")
    out_d = nc.dram_tensor("out", [128, R5], F32, kind="ExternalOutput")

    Relu = mybir.ActivationFunctionType.Relu

    with TileContext(nc) as tc:
        with tc.tile_pool(name="sb", bufs=1) as pool, \
             tc.tile_pool(name="ps", bufs=3, space="PSUM") as pp, \
             tc.tile_pool(name="ps2", bufs=5, space="PSUM") as pp2:
            x1t = pool.tile([128, 4 * CHK1], BF16, tag="x1")
            w1t = pool.tile([128, 6 * 128], BF16, tag="w1")
            # x2 store: [1 lead margin][R1 blocks of BLK][91 tail margin]
            x2t = pool.tile([128, X2N + 92], BF16, tag="x2")
            w2at = pool.tile([128, 27 * 128], BF16, tag="w2a")
            w2bt = pool.tile([128, 27 * 128], BF16, tag="w2b")
            w2pt = pool.tile([128, 9 * 128], BF16, tag="w2p")
            # second shifted tile: parts 0-63 = x2, parts 64-127 = x2[+10],
            # pairing (j3, j3+1) taps of the j4=2 pass into K=128 matmuls
            x2st = pool.tile([128, X2N + 92], BF16, tag="x2s")
            bpt = pool.tile([128, 6], F32, tag="bp")
            x3t = pool.tile([128, N3], BF16, tag="x3")
            wpt = pool.tile([128, 8 * 128], BF16, tag="wp")
            x4t = pool.tile([128, 2 * N3], BF16, tag="x4")
            x4bt = pool.tile([128, 2 * N3], BF16, tag="x4b")
            x5t = pool.tile([128, N5], F32, tag="x5")
            fcwt = pool.tile([128, N5], F32, tag="fcw")
            prodt = pool.tile([128, N5], F32, tag="prod")
            fct = pool.tile([128, R5], F32, tag="fc")
            b2t = bpt[:, 0:1]
            b3t = bpt[:, 1:3]
            b4t = bpt[:, 3:5]
            b5t = bpt[:, 5:6]
            w3t = wpt[:, 0:256]
            w4t = wpt[:, 256:768]
            w5t = wpt[:, 768:1024]

            # --- weight / bias loads (w1 first: conv1 needs it sooner) ---
            nc.sync.dma_start(w1t[:, :], w1_d[:, :])

            def load_x1(c, half):
                # chunk c carries conv1 rows c (parts 0-63) and c+4 (64-127);
                # halves split at the o2 4/5 boundary so each conv1 unit
                # starts as soon as its own half lands
                lo = c * CHK1 + (0 if half == 0 else 5 * 171)
                hi = c * CHK1 + (5 * 171 if half == 0 else CHK1)
                nc.sync.dma_start(x1t[:, lo:hi], x1_d[:, lo:hi])

            # x1 chunks first (conv2's first group needs conv1 rows 0..3),
            # then w2a in consumption-ordered thirds, then the rest.
            for c in range(4):
                load_x1(c, 0)
                load_x1(c, 1)
            for c in range(3):
                nc.sync.dma_start(w2at[:, c * 1152:(c + 1) * 1152],
                                  w2a_d[:, c * 1152:(c + 1) * 1152])
            nc.sync.dma_start(bpt[:, :], bp_d[:, :])

            # --- one-time zero fills ---
            nc.vector.memset(w2bt[64:128, :], 0)
            # x2 lead margin + tail margin (both partition halves)
            nc.vector.memset(x2t[:, 0:1], 0)
            nc.vector.memset(x2t[:, 1 + X2N:X2N + 92], 0)
            # d4 pad column of every (block, d2, d3) row
            x2pad = x2t[:, 1:1 + X2N].rearrange("p (x c) -> p x c", c=D4P)
            nc.vector.memset(x2pad[:, :, 9:10], 0)

            s1v = x1t.rearrange("p (r a b c) -> p r a b c", r=4, a=9, b=9, c=D4S)
            x2v = x2t[:, 1:1 + X2N].rearrange(
                "p (r a b c) -> p r a b c", r=R1, a=9, b=9, c=D4P)
            x3v = x3t.rearrange("p (r a b c) -> p r a b c", r=R2, a=9, b=9, c=9)
            x4bv = x4bt.rearrange("p (m r a b c) -> p m r a b c",
                                  m=2, r=R2, a=9, b=9, c=9)
            chunks = []
            pos = 0
            while pos < N3:
                sz = min(512, N3 - pos)
                chunks.append((pos, sz))
                pos += sz

            for rep in range(reps):
                if rep > 0:
                    # steady-state reload of the activations slab
                    for c in range(4):
                        load_x1(c, 0)
                        load_x1(c, 1)

                # ---------------- conv1 + shifted-replica build ----------------
                def conv1_unit(r):
                    v, rc = divmod(r, 4)
                    for gi, (o2s, c2g) in enumerate(((0, 5), (5, 4))):
                        n = c2g * 81
                        ps1 = pp.tile([128, 512], F32, tag="ps")
                        ps1v = ps1[:, :n].rearrange("p (a b c) -> p a b c",
                                                    a=c2g, b=9, c=9)
                        for j4 in range(3):
                            rhs = s1v[:, rc, o2s:o2s + c2g, :, j4:j4 + 17:2]
                            nc.tensor.matmul(
                                ps1v[:, :, :, :],
                                w1t[:, (v * 3 + j4) * 128:
                                       (v * 3 + j4 + 1) * 128],
                                rhs,
                                start=(j4 == 0), stop=(j4 == 2))
                        dst = x2v[0:64, r, o2s:o2s + c2g, :, 0:9]
                        src = ps1v[0:64, :, :, :]
                        # evacuate on alternating engines so the ACT chain
                        # doesn't delay the replica copies
                        if gi == 0:
                            nc.scalar.activation(dst, src, Relu)
                        else:
                            nc.vector.tensor_scalar(dst, src, 0.0, None,
                                                    mybir.AluOpType.max)

                def replica(r):
                    # shifted copy of block r (needs first elem of block r+1;
                    # the last block reads one col into the tail margin)
                    b0 = r * BLK
                    hi = b0 + BLK if r < R1 - 1 else X2N + 1
                    nc.sync.dma_start(x2t[64:128, b0:hi],
                                      x2t[0:64, b0 + 1:hi + 1])
                    # x2s block: lower half verbatim, upper half shifted +10
                    hi2_ = b0 + BLK if r < R1 - 1 else X2N + 82
                    nc.sync.dma_start(x2st[0:64, b0:hi2_],
                                      x2t[0:64, b0:hi2_])
                    nc.sync.dma_start(x2st[64:128, b0:hi2_],
                                      x2t[0:64, b0 + 10:hi2_ + 10])

                # row r lands with chunk r%4; conv2 runs its groups r=4..0 so
                # emit high rows/replicas first to match chunk arrival
                conv1_unit(4); conv1_unit(0)
                conv1_unit(5); conv1_unit(1)
                replica(4)
                conv1_unit(6); conv1_unit(2)
                replica(5); replica(6)
                conv1_unit(3)
                replica(3); replica(2); replica(1); replica(0)
                if rep == 0:
                    for c in range(3):
                        nc.sync.dma_start(w2bt[0:64, c * 1152:(c + 1) * 1152],
                                          w2b_d[:, c * 1152:(c + 1) * 1152])
                    nc.sync.dma_start(w2pt[:, :], w2p_d[:, :])

                # ---------------- conv2 ----------------
                # taps restricted to the valid (o2, o3) window; returns the
                # PSUM out view and the x2 flat base of the window start
                def tap_geom(ps2v, r, o2s, c2g, j1, j2, j3):
                    blk = r + j1
                    lo2 = max(o2s, 1 - j2)
                    hi2 = min(o2s + c2g, 10 - j2)
                    lo3 = max(0, 1 - j3)
                    hi3 = min(9, 10 - j3)
                    c2 = hi2 - lo2
                    c3 = hi3 - lo3
                    out_ap = ps2v[:, lo2 - o2s:hi2 - o2s, lo3:hi3, :]
                    base0 = (blk * BLK + (lo2 + j2 - 1) * 90
                             + (lo3 + j3 - 1) * D4P)
                    return out_ap, base0, c2, c3

                def rhs_win(plo, phi, base, c2, c3):
                    return x2t[plo:phi, base:base + c2 * 90].rearrange(
                        "p (a b c) -> p a b c", a=c2, b=9, c=D4P)[
                        :, :, 0:c3, 0:9]

                for r in range(R2 - 1, -1, -1):
                    # both o2s-subgroups batched per pass so the PE tile mode
                    # toggles once per direction, not per tap
                    groups = []
                    for (o2s, c2g) in ((0, 5), (5, 4)):
                        n = c2g * 81
                        ps2 = pp.tile([128, 512], F32, tag="ps")
                        groups.append((ps2[:, :n].rearrange(
                            "p (a b c) -> p a b c", a=c2g, b=9, c=9),
                            o2s, c2g))
                    # pass 1: 27 j4-fused K=128 taps (interior first)
                    for (ps2v, o2s, c2g) in groups:
                        for ti, (j1, j2, j3) in enumerate(TAPS):
                            out_ap, base0, c2, c3 = tap_geom(
                                ps2v, r, o2s, c2g, j1, j2, j3)
                            nc.tensor.matmul(
                                out_ap,
                                w2at[:, ti * 128:(ti + 1) * 128],
                                rhs_win(0, 128, base0, c2, c3),
                                start=(ti == 0), stop=False)
                    # pass 2 (j4=2 taps): for each (j1,j2), taps j3=0 and
                    # j3=1 run as one dense K=128 matmul against x2st over
                    # the intersection o3 in [1,9) (x2st upper partitions
                    # hold x2[+10] = the j3+1 window); the j3=1 tap's o3=0
                    # sliver and the j3=2 tap run K zero-padded to 128.
                    for (ps2v, o2s, c2g) in groups:
                        for q, (j1, j2) in enumerate(
                                itertools.product(range(3), range(3))):
                            blk = r + j1
                            lo2 = max(o2s, 1 - j2)
                            hi2 = min(o2s + c2g, 10 - j2)
                            c2 = hi2 - lo2
                            d2base = blk * BLK + (lo2 + j2 - 1) * 90
                            # pair (j3=0 rows 0-63, j3=1 rows 64-127)
                            bp_ = d2base + 0 * D4P + 2
                            rhsp = x2st[:, bp_:bp_ + c2 * 90].rearrange(
                                "p (a b c) -> p a b c", a=c2, b=9, c=D4P)[
                                :, :, 0:8, 0:9]
                            nc.tensor.matmul(
                                ps2v[:, lo2 - o2s:hi2 - o2s, 1:9, :],
                                w2pt[:, q * 128:(q + 1) * 128],
                                rhsp,
                                start=False, stop=False)
                            # sliver: tap (j1,j2,1) at o3=0
                            bs_ = d2base + 0 * D4P + 2
                            rhss = x2t[:, bs_:bs_ + c2 * 90].rearrange(
                                "p (a b c) -> p a b c", a=c2, b=9, c=D4P)[
                                :, :, 0:1, 0:9]
                            tb = TI[(j1, j2, 1)]
                            nc.tensor.matmul(
                                ps2v[:, lo2 - o2s:hi2 - o2s, 0:1, :],
                                w2bt[:, tb * 128:(tb + 1) * 128],
                                rhss,
                                start=False, stop=False)
                            # single: tap (j1,j2,2), o3 in [0,8)
                            bc_ = d2base + 1 * D4P + 2
                            rhsc = x2t[:, bc_:bc_ + c2 * 90].rearrange(
                                "p (a b c) -> p a b c", a=c2, b=9, c=D4P)[
                                :, :, 0:8, 0:9]
                            tcq = TI[(j1, j2, 2)]
                            nc.tensor.matmul(
                                ps2v[:, lo2 - o2s:hi2 - o2s, 0:8, :],
                                w2bt[:, tcq * 128:(tcq + 1) * 128],
                                rhsc,
                                start=False, stop=(q == 8))
                    for gi, (ps2v, o2s, c2g) in enumerate(groups):
                        dst = x3v[:, r, o2s:o2s + c2g, :, :]
                        if gi == 0:
                            nc.scalar.activation(dst, ps2v[:, :, :, :],
                                                 Relu, bias=b2t[:, :])
                        else:
                            nc.vector.tensor_scalar(
                                dst, ps2v[:, :, :, :], b2t[:, :], 0.0,
                                mybir.AluOpType.add, mybir.AluOpType.max)

                if rep == 0:
                    # late weights: emitted after conv2 so the x2 replica
                    # copies outrank them on the DMA queue; they still land
                    # long before conv3 needs them.
                    nc.sync.dma_start(wpt[:, :], wp_d[:, :])
                    nc.sync.dma_start(fcwt[:, :], fcw_d[:, :])

                # ---------------- conv3 + conv4 (1x1), chunk-interleaved ----
                Amax = mybir.AluOpType.max
                Aadd = mybir.AluOpType.add

                def evac(dst, src, bias, mh):
                    # PSUM evacuation alternates engines: ScalarE handles
                    # mh=0, VectorE mh=1, so neither engine serializes PE.
                    if mh == 0:
                        nc.scalar.activation(dst, src, Relu, bias=bias)
                    else:
                        nc.vector.tensor_scalar(dst, src, bias, 0.0,
                                                Aadd, Amax)

                def conv3_chunk(pos, sz):
                    for mh in range(2):
                        ps3 = pp2.tile([128, 512], F32, tag="ps2")
                        nc.tensor.matmul(
                            ps3[:, :sz],
                            w3t[:, mh * 128:(mh + 1) * 128],
                            x3t[:, pos:pos + sz],
                            start=True, stop=True)
                        evac(x4t[:, mh * N3 + pos:mh * N3 + pos + sz],
                             ps3[:, :sz], b3t[:, mh:mh + 1], mh)

                def conv4_chunk(pos, sz):
                    for mh in range(2):
                        ps4 = pp2.tile([128, 512], F32, tag="ps2")
                        nc.tensor.matmul(
                            ps4[:, :sz],
                            w4t[:, (mh * 2) * 128:(mh * 2 + 1) * 128],
                            x4t[:, pos:pos + sz],
                            start=True, stop=False)
                        nc.tensor.matmul(
                            ps4[:, :sz],
                            w4t[:, (mh * 2 + 1) * 128:(mh * 2 + 2) * 128],
                            x4t[:, N3 + pos:N3 + pos + sz],
                            start=False, stop=True)
                        evac(x4bt[:, mh * N3 + pos:mh * N3 + pos + sz],
                             ps4[:, :sz], b4t[:, mh:mh + 1], mh)

                # two-chunk software pipeline: conv4(k) trails conv3(k+2)
                rchunks = list(reversed(chunks))
                conv3_chunk(*rchunks[0])
                conv3_chunk(*rchunks[1])
                for i in range(len(rchunks)):
                    if i + 2 < len(rchunks):
                        conv3_chunk(*rchunks[i + 2])
                    conv4_chunk(*rchunks[i])

                # ---------------- conv5 (1x1, s=2, 128c->64c) ----------------
                for rr in range(R5 - 1, -1, -1):
                    ps5 = pp2.tile([128, 512], F32, tag="ps2")
                    for mb in range(2):
                        rhs = x4bv[:, mb, 2 * rr, 0:9:2, 0:9:2, 0:9:2]
                        nc.tensor.matmul(
                            ps5[:, :125],
                            w5t[:, mb * 128:(mb + 1) * 128],
                            rhs,
                            start=(mb == 0), stop=(mb == 1))
                    nc.scalar.activation(
                        x5t[:, rr * 125:(rr + 1) * 125],
                        ps5[:, :125],
                        Relu, bias=b5t[:, :])
                    nc.vector.tensor_mul(
                        prodt[:, rr * 125:(rr + 1) * 125],
                        x5t[:, rr * 125:(rr + 1) * 125],
                        fcwt[:, rr * 125:(rr + 1) * 125])
                    nc.vector.reduce_sum(
                        fct[:, rr:rr + 1],
                        prodt[:, rr * 125:(rr + 1) * 125],
                        axis=mybir.AxisListType.X)


            nc.sync.dma_start(out_d[:, :], fct[:, :])

    nc.compile()
    return nc


# ---------------- host-side data prep ----------------

def _prep_weights(inputs):
    f32 = np.float32
    w1r = np.asarray(inputs["w1r"], f32)[:, 0]   # [32, 3,3,3,3]
    w1i = np.asarray(inputs["w1i"], f32)[:, 0]
    # [t27, j4, co]
    w1r_t = w1r.transpose(1, 2, 3, 4, 0).reshape(27, 3, 32)
    w1i_t = w1i.transpose(1, 2, 3, 4, 0).reshape(27, 3, 32)
    # two variants: cols [0:384) contract slab rows 0-3 (K rows 0-54),
    # cols [384:768) contract slab rows 4-6 (K rows 64-118)
    W1 = np.zeros((128, 6 * 128), f32)
    for v in range(2):
        k0 = 64 * v
        for j4 in range(3):
            c0 = (v * 3 + j4) * 128
            W1[k0 + 0:k0 + 27, c0:c0 + 32] = w1r_t[:, j4]
            W1[k0 + 0:k0 + 27, c0 + 32:c0 + 64] = w1i_t[:, j4]
            W1[k0 + 27:k0 + 54, c0:c0 + 32] = -w1i_t[:, j4]
            W1[k0 + 27:k0 + 54, c0 + 32:c0 + 64] = w1r_t[:, j4]
        W1[k0 + 54, v * 3 * 128:v * 3 * 128 + 32] = np.asarray(inputs["b1r"], f32)
        W1[k0 + 54, v * 3 * 128 + 32:v * 3 * 128 + 64] = np.asarray(inputs["b1i"], f32)

    w2r = np.asarray(inputs["w2r"], f32)   # [64, 32, 3,3,3,3]
    w2i = np.asarray(inputs["w2i"], f32)
    # [t27, j4, ci, co]
    w2r_t = w2r.transpose(2, 3, 4, 5, 1, 0).reshape(27, 3, 32, 64)
    w2i_t = w2i.transpose(2, 3, 4, 5, 1, 0).reshape(27, 3, 32, 64)
    W2a = np.zeros((128, 27 * 128), f32)
    W2b = np.zeros((64, 27 * 128), f32)
    # columns packed in TAPS (kernel emission) order
    for ti, (j1, j2, j3) in enumerate(TAPS):
        t = j1 * 9 + j2 * 3 + j3
        for jj, r0 in ((0, 0), (1, 64)):
            W2a[r0 + 0:r0 + 32, ti * 128:ti * 128 + 64] = w2r_t[t, jj]
            W2a[r0 + 0:r0 + 32, ti * 128 + 64:(ti + 1) * 128] = w2i_t[t, jj]
            W2a[r0 + 32:r0 + 64, ti * 128:ti * 128 + 64] = -w2i_t[t, jj]
            W2a[r0 + 32:r0 + 64, ti * 128 + 64:(ti + 1) * 128] = w2r_t[t, jj]
        W2b[0:32, ti * 128:ti * 128 + 64] = w2r_t[t, 2]
        W2b[0:32, ti * 128 + 64:(ti + 1) * 128] = w2i_t[t, 2]
        W2b[32:64, ti * 128:ti * 128 + 64] = -w2i_t[t, 2]
        W2b[32:64, ti * 128 + 64:(ti + 1) * 128] = w2r_t[t, 2]
    # j3-pair weights for the j4=2 pass: block q=(j1,j2) holds tap
    # (j1,j2,0) on K rows 0-63 and tap (j1,j2,1) on rows 64-127
    W2p = np.zeros((128, 9 * 128), f32)
    for q, (j1, j2) in enumerate(itertools.product(range(3), range(3))):
        for j3, k0 in ((0, 0), (1, 64)):
            t = j1 * 9 + j2 * 3 + j3
            W2p[k0 + 0:k0 + 32, q * 128:q * 128 + 64] = w2r_t[t, 2]
            W2p[k0 + 0:k0 + 32, q * 128 + 64:(q + 1) * 128] = w2i_t[t, 2]
            W2p[k0 + 32:k0 + 64, q * 128:q * 128 + 64] = -w2i_t[t, 2]
            W2p[k0 + 32:k0 + 64, q * 128 + 64:(q + 1) * 128] = w2r_t[t, 2]
    B2 = np.concatenate([np.asarray(inputs["b2r"], f32),
                         np.asarray(inputs["b2i"], f32)])[:, None]

    w3r = np.asarray(inputs["w3r"], f32).reshape(128, 64)
    w3i = np.asarray(inputs["w3i"], f32).reshape(128, 64)
    W3 = np.zeros((128, 2 * 128), f32)
    W3[0:64, 0:128] = w3r.T
    W3[64:128, 0:128] = -w3i.T
    W3[0:64, 128:256] = w3i.T
    W3[64:128, 128:256] = w3r.T
    B3 = np.stack([np.asarray(inputs["b3r"], f32),
                   np.asarray(inputs["b3i"], f32)], axis=1)

    w4r = np.asarray(inputs["w4r"], f32).reshape(128, 128)
    w4i = np.asarray(inputs["w4i"], f32).reshape(128, 128)
    W4 = np.zeros((128, 4 * 128), f32)
    W4[:, 0:128] = w4r.T
    W4[:, 128:256] = -w4i.T
    W4[:, 256:384] = w4i.T
    W4[:, 384:512] = w4r.T
    B4 = np.stack([np.asarray(inputs["b4r"], f32),
                   np.asarray(inputs["b4i"], f32)], axis=1)

    w5r = np.asarray(inputs["w5r"], f32).reshape(64, 128)
    w5i = np.asarray(inputs["w5i"], f32).reshape(64, 128)
    W5 = np.zeros((128, 2 * 128), f32)
    W5[:, 0:64] = w5r.T
    W5[:, 64:128] = w5i.T
    W5[:, 128:192] = -w5i.T
    W5[:, 192:256] = w5r.T
    B5 = np.concatenate([np.asarray(inputs["b5r"], f32),
                         np.asarray(inputs["b5i"], f32)])[:, None]

    BP = np.zeros((128, 6), f32)
    BP[:, 0:1] = B2
    BP[:, 1:3] = B3
    BP[:, 3:5] = B4
    BP[:, 5:6] = B5
    WP = np.zeros((128, 8 * 128), f32)
    WP[:, 0:256] = W3
    WP[:, 256:768] = W4
    WP[:, 768:1024] = W5
    return {
        "w1": W1.astype(BF), "w2a": W2a.astype(BF), "w2b": W2b.astype(BF),
        "w2p": W2p.astype(BF), "bp": BP, "wp": WP.astype(BF),
    }


def _prep_x1(xr_b, xi_b, h):
    """Conv1 input slab for one (batch, half), packed [128, 4*CHK1] bf16:
    partition p < 64 holds slab row r=chunk of tap p; partition 64+p holds
    row chunk+4 (row 7 slot is zero)."""
    S = np.zeros((64, R1 + 1, 9, 9, D4S), np.float32)
    glo = max(0, 4 * h - 1)
    ghi = min(8, 4 * h + 5)
    rlo = glo - (4 * h - 1)
    rhi = ghi - (4 * h - 1) + 1
    for t, (j1, j2, j3) in enumerate(itertools.product(range(3), repeat=3)):
        subr = xr_b[j1:j1 + 17:2, j2:j2 + 17:2, j3:j3 + 17:2, :D4S]
        subi = xi_b[j1:j1 + 17:2, j2:j2 + 17:2, j3:j3 + 17:2, :D4S]
        S[t, rlo:rhi] = subr[glo:ghi + 1]
        S[27 + t, rlo:rhi] = subi[glo:ghi + 1]
    S[54, rlo:rhi] = 1.0
    S = S.reshape(64, 2, 4 * CHK1)
    return np.concatenate([S[:, 0], S[:, 1]], axis=0).astype(BF)


def _prep_fcw(fcw, h):
    out = np.zeros((128, N5), np.float32)
    f = np.asarray(fcw, np.float32).reshape(-1)
    for rr in range(R5):
        g5 = rr + 2 * h
        if h == 1 and rr == 0:
            continue  # overlap row masked on half 1
        out[:, rr * 125:(rr + 1) * 125] = f[g5 * 125:(g5 + 1) * 125][None, :]
    return out


def _make_in_maps(inputs):
    wkey = id(inputs.get("w1r"))
    if _CACHE.get("wkey") != wkey:
        _CACHE["wmaps"] = _prep_weights(inputs)
        _CACHE["wkey"] = wkey
    wmaps = _CACHE["wmaps"]
    xr = np.asarray(inputs["xr"], np.float32)
    xi = np.asarray(inputs["xi"], np.float32)
    fcw = inputs["fcw"]
    in_maps = []
    for core in range(8):
        b, h = core // 2, core % 2
        m = dict(wmaps)
        m["x1"] = _prep_x1(xr[b, 0], xi[b, 0], h)
        m["fcw"] = _prep_fcw(fcw, h)
        in_maps.append(m)
    return in_maps


def kernel(**inputs):
    if "nc" not in _CACHE:
        _CACHE["nc"] = _build_nc()
    nc = _CACHE["nc"]

    in_maps = _make_in_maps(inputs)
    res = run_bass_kernel_spmd(nc, in_maps, core_ids=list(range(8)))

    fcb = np.asarray(inputs["fcb"], np.float32)
    yr = np.zeros((NB, 64, 1), np.float32)
    yi = np.zeros((NB, 64, 1), np.float32)
    for b in range(NB):
        p0 = res.results[2 * b]["out"]
        p1 = res.results[2 * b + 1]["out"]
        s = (p0 + p1).sum(axis=1, keepdims=True)
        yr[b] = s[0:64] + fcb[0]
        yi[b] = s[64:128]
    return np.stack([yr, yi]).astype(np.float32)


# revision 23
# speedup vs baseline: 1.1410x; 1.0207x over previous
"""Trainium2 Bass kernel for nn_LASLNNet (complex-valued 4D CNN).

Strategy (8 NeuronCores, SPMD single program):
  - core c handles (batch b = c//2, spatial half h = c%2) -> 4 x 2 split.
  - All complex convs are computed as real matmuls with doubled channels:
      [yr; yi] = [[Wr, Wi], [-Wi, Wr]]^T @ [xr; xi]
  - conv1 (k=3,s=2): im2col-lite slabs prepared on host (27 (j1,j2,j3) tap
    slabs; j4 handled as 3 PSUM-accumulated matmuls with step-2 rhs reads).
    Bias folded in via an all-ones K-channel so dummy edge rows stay zero.
    The slab DMA is split into R1 per-row chunks so conv1 row r starts as
    soon as chunk r lands; K and M are zero-padded to 128 so every matmul
    in the kernel runs in the same (128,128) PE tile mode.
  - conv2 (k=3,s=1,p=1): input stored on a d4-padded flat grid
    [block(d1) 7, d2 9, d3 9, d4 10] so each (j1,j2,j3) tap is a single
    flat offset; j4 in {0,1} fused into one K=128 matmul via a 1-element
    shifted replica of the input on partitions 64..127; j4=2 is a K=64
    matmul zero-padded to K=128. The replica is built with per-d1-block
    SBUF->SBUF DMAs that overlap conv1 compute (no bulk barrier).
    Edge taps restrict (o2,o3) ranges via strided APs; PSUM has_written
    semantics make partial-region accumulation correct (the first matmul
    of each group is the full-region interior tap). Within each PSUM
    group all 27 K=128 j4-fused taps are issued first, then the 27 j4=2
    taps, so the PE tile configuration never toggles mid-group.
  - conv3/4 (1x1): plain matmuls on a compact layout, interleaved per
    512-column chunk (conv3 both halves then conv4 both halves) so conv4
    starts while conv3 still runs on later chunks.
  - conv5 (1x1,s=2) then FC: on-chip mul+reduce against host-sliced fcw;
    final cross-half sum + fc bias on host (each core returns [128,1]).
  - dtype: bf16 matmul operands, fp32 PSUM/copies.

Spatial split along first output spatial dim D1 (9 rows):
  half 0 -> conv2..4 rows 0..4, half 1 -> rows 4..8 (row 4 duplicated);
  conv5 rows {0,1,2} / {2,3,4} (row 2 duplicated, masked via zeroed fcw).

_build_nc(reps=N) unrolls the steady-state body (x1 reload + all compute)
N times inside one NEFF; test.py uses (T(reps)-T(1))/(reps-1) to measure
the true per-iteration HW execution time independent of dispatch latency.
"""

import itertools

import numpy as np
import ml_dtypes

import concourse.bacc as bacc
import concourse.mybir as mybir
from concourse.tile import TileContext
from concourse.bass_utils import run_bass_kernel_spmd

F32 = mybir.dt.float32
BF16 = mybir.dt.bfloat16
BF = ml_dtypes.bfloat16

NB = 4            # batch
R1 = 7            # conv1 rows computed per core (incl. dummy edge rows)
R2 = 5            # conv2/3/4 rows per core
R5 = 3            # conv5 rows per core
D4P = 10          # d4-padded inner dim (9 valid + 1 zero)
D4S = 19          # raw d4 columns kept in the conv1 slab (col 19 never read)
BLK = 9 * 9 * D4P                # 810, one d1-block of x2
X2N = R1 * BLK                   # logical x2 elements per partition
CHK1 = 9 * 9 * D4S               # 1539, one conv1 slab row chunk
S1N = R1 * CHK1                  # conv1 slab elements per partition
# conv2 taps ordered interior-first so the first matmul of each PSUM group
# covers the full region (has_written correctness); weights are packed on
# the host in this same order so chunked weight DMAs land in consumption
# order.
TAPS = sorted(itertools.product(range(3), repeat=3),
              key=lambda t: (t != (1, 1, 1)))
TI = {t: i for i, t in enumerate(TAPS)}
N3 = R2 * 729                    # 3645 compact columns for conv3/4
N5 = R5 * 125                    # 375 conv5 output columns

_CACHE = {}


def _build_nc(reps=1):
    nc = bacc.Bacc("TRN2", target_bir_lowering=False, debug=False)

    x1_d = nc.dram_tensor("x1", [128, 4 * CHK1], BF16, kind="ExternalInput")
    w1_d = nc.dram_tensor("w1", [128, 6 * 128], BF16, kind="ExternalInput")
    w2a_d = nc.dram_tensor("w2a", [128, 27 * 128], BF16, kind="ExternalInput")
    w2b_d = nc.dram_tensor("w2b", [64, 27 * 128], BF16, kind="ExternalInput")
    w2p_d = nc.dram_tensor("w2p", [128, 9 * 128], BF16, kind="ExternalInput")
    w2q_d = nc.dram_tensor("w2q", [128, 3 * 128], BF16, kind="ExternalInput")
    bp_d = nc.dram_tensor("bp", [128, 6], F32, kind="ExternalInput")
    wp_d = nc.dram_tensor("wp", [128, 8 * 128], BF16, kind="ExternalInput")
    fcw_d = nc.dram_tensor("fcw", [128, N5], F32, kind="ExternalInput")
    out_d = nc.dram_tensor("out", [128, R5], F32, kind="ExternalOutput")

    Relu = mybir.ActivationFunctionType.Relu

    with TileContext(nc) as tc:
        with tc.tile_pool(name="sb", bufs=1) as pool, \
             tc.tile_pool(name="ps", bufs=3, space="PSUM") as pp, \
             tc.tile_pool(name="ps2", bufs=5, space="PSUM") as pp2:
            x1t = pool.tile([128, 4 * CHK1], BF16, tag="x1")
            w1t = pool.tile([128, 6 * 128], BF16, tag="w1")
            # x2 store: [1 lead margin][R1 blocks of BLK][91 tail margin]
            x2t = pool.tile([128, X2N + 92], BF16, tag="x2")
            w2at = pool.tile([128, 27 * 128], BF16, tag="w2a")
            w2bt = pool.tile([128, 27 * 128], BF16, tag="w2b")
            w2pt = pool.tile([128, 9 * 128], BF16, tag="w2p")
            # second shifted tile: parts 0-63 = x2, parts 64-127 = x2[+10],
            # pairing (j3, j3+1) taps of the j4=2 pass into K=128 matmuls
            x2st = pool.tile([128, X2N + 92], BF16, tag="x2s")
            w2qt = pool.tile([128, 3 * 128], BF16, tag="w2q")
            # third shifted tile: parts 0-63 = x2, parts 64-127 = x2[+90],
            # pairing (j2, j2+1) among the j3=2, j4=2 leftover taps
            x2ut = pool.tile([128, X2N + 92], BF16, tag="x2u")
            bpt = pool.tile([128, 6], F32, tag="bp")
            x3t = pool.tile([128, N3], BF16, tag="x3")
            wpt = pool.tile([128, 8 * 128], BF16, tag="wp")
            x4t = pool.tile([128, 2 * N3], BF16, tag="x4")
            x4bt = pool.tile([128, 2 * N3], BF16, tag="x4b")
            x5t = pool.tile([128, N5], F32, tag="x5")
            fcwt = pool.tile([128, N5], F32, tag="fcw")
            prodt = pool.tile([128, N5], F32, tag="prod")
            fct = pool.tile([128, R5], F32, tag="fc")
            b2t = bpt[:, 0:1]
            b3t = bpt[:, 1:3]
            b4t = bpt[:, 3:5]
            b5t = bpt[:, 5:6]
            w3t = wpt[:, 0:256]
            w4t = wpt[:, 256:768]
            w5t = wpt[:, 768:1024]

            # --- weight / bias loads (w1 first: conv1 needs it sooner) ---
            nc.sync.dma_start(w1t[:, :], w1_d[:, :])

            def load_x1(c, half):
                # chunk c carries conv1 rows c (parts 0-63) and c+4 (64-127);
                # halves split at the o2 4/5 boundary so each conv1 unit
                # starts as soon as its own half lands
                lo = c * CHK1 + (0 if half == 0 else 5 * 171)
                hi = c * CHK1 + (5 * 171 if half == 0 else CHK1)
                nc.sync.dma_start(x1t[:, lo:hi], x1_d[:, lo:hi])

            # x1 chunks first (conv2's first group needs conv1 rows 0..3),
            # then w2a in consumption-ordered thirds, then the rest.
            for c in range(4):
                load_x1(c, 0)
                load_x1(c, 1)
            for c in range(3):
                nc.sync.dma_start(w2at[:, c * 1152:(c + 1) * 1152],
                                  w2a_d[:, c * 1152:(c + 1) * 1152])
            nc.sync.dma_start(bpt[:, :], bp_d[:, :])

            # --- one-time zero fills ---
            nc.vector.memset(w2bt[64:128, :], 0)
            # x2 lead margin + tail margin (both partition halves)
            nc.vector.memset(x2t[:, 0:1], 0)
            nc.vector.memset(x2t[:, 1 + X2N:X2N + 92], 0)
            # d4 pad column of every (block, d2, d3) row
            x2pad = x2t[:, 1:1 + X2N].rearrange("p (x c) -> p x c", c=D4P)
            nc.vector.memset(x2pad[:, :, 9:10], 0)

            s1v = x1t.rearrange("p (r a b c) -> p r a b c", r=4, a=9, b=9, c=D4S)
            x2v = x2t[:, 1:1 + X2N].rearrange(
                "p (r a b c) -> p r a b c", r=R1, a=9, b=9, c=D4P)
            x3v = x3t.rearrange("p (r a b c) -> p r a b c", r=R2, a=9, b=9, c=9)
            x4bv = x4bt.rearrange("p (m r a b c) -> p m r a b c",
                                  m=2, r=R2, a=9, b=9, c=9)
            chunks = []
            pos = 0
            while pos < N3:
                sz = min(512, N3 - pos)
                chunks.append((pos, sz))
                pos += sz

            for rep in range(reps):
                if rep > 0:
                    # steady-state reload of the activations slab
                    for c in range(4):
                        load_x1(c, 0)
                        load_x1(c, 1)

                # ---------------- conv1 + shifted-replica build ----------------
                def conv1_unit(r):
                    v, rc = divmod(r, 4)
                    for gi, (o2s, c2g) in enumerate(((0, 5), (5, 4))):
                        n = c2g * 81
                        ps1 = pp.tile([128, 512], F32, tag="ps")
                        ps1v = ps1[:, :n].rearrange("p (a b c) -> p a b c",
                                                    a=c2g, b=9, c=9)
                        for j4 in range(3):
                            rhs = s1v[:, rc, o2s:o2s + c2g, :, j4:j4 + 17:2]
                            nc.tensor.matmul(
                                ps1v[:, :, :, :],
                                w1t[:, (v * 3 + j4) * 128:
                                       (v * 3 + j4 + 1) * 128],
                                rhs,
                                start=(j4 == 0), stop=(j4 == 2))
                        dst = x2v[0:64, r, o2s:o2s + c2g, :, 0:9]
                        src = ps1v[0:64, :, :, :]
                        # evacuate on alternating engines so the ACT chain
                        # doesn't delay the replica copies
                        if gi == 0:
                            nc.scalar.activation(dst, src, Relu)
                        else:
                            nc.vector.tensor_scalar(dst, src, 0.0, None,
                                                    mybir.AluOpType.max)

                def replica(r):
                    # shifted copy of block r (needs first elem of block r+1;
                    # the last block reads one col into the tail margin)
                    b0 = r * BLK
                    hi = b0 + BLK if r < R1 - 1 else X2N + 1
                    nc.sync.dma_start(x2t[64:128, b0:hi],
                                      x2t[0:64, b0 + 1:hi + 1])
                    # x2s block: lower half verbatim, upper half shifted +10
                    hi2_ = b0 + BLK if r < R1 - 1 else X2N + 82
                    nc.sync.dma_start(x2st[0:64, b0:hi2_],
                                      x2t[0:64, b0:hi2_])
                    nc.sync.dma_start(x2st[64:128, b0:hi2_],
                                      x2t[0:64, b0 + 10:hi2_ + 10])
                    # x2u block: lower half verbatim, upper half shifted +90
                    hi3_ = b0 + BLK if r < R1 - 1 else X2N + 2
                    nc.sync.dma_start(x2ut[0:64, b0:hi3_],
                                      x2t[0:64, b0:hi3_])
                    nc.sync.dma_start(x2ut[64:128, b0:hi3_],
                                      x2t[0:64, b0 + 90:hi3_ + 90])

                # row r lands with chunk r%4; conv2 runs its groups r=4..0 so
                # emit high rows/replicas first to match chunk arrival
                conv1_unit(4); conv1_unit(0)
                conv1_unit(5); conv1_unit(1)
                replica(4)
                conv1_unit(6); conv1_unit(2)
                replica(5); replica(6)
                conv1_unit(3)
                replica(3); replica(2); replica(1); replica(0)
                if rep == 0:
                    for c in range(3):
                        nc.sync.dma_start(w2bt[0:64, c * 1152:(c + 1) * 1152],
                                          w2b_d[:, c * 1152:(c + 1) * 1152])
                    nc.sync.dma_start(w2pt[:, :], w2p_d[:, :])
                    nc.sync.dma_start(w2qt[:, :], w2q_d[:, :])

                # ---------------- conv2 ----------------
                # taps restricted to the valid (o2, o3) window; returns the
                # PSUM out view and the x2 flat base of the window start
                def tap_geom(ps2v, r, o2s, c2g, j1, j2, j3):
                    blk = r + j1
                    lo2 = max(o2s, 1 - j2)
                    hi2 = min(o2s + c2g, 10 - j2)
                    lo3 = max(0, 1 - j3)
                    hi3 = min(9, 10 - j3)
                    c2 = hi2 - lo2
                    c3 = hi3 - lo3
                    out_ap = ps2v[:, lo2 - o2s:hi2 - o2s, lo3:hi3, :]
                    base0 = (blk * BLK + (lo2 + j2 - 1) * 90
                             + (lo3 + j3 - 1) * D4P)
                    return out_ap, base0, c2, c3

                def rhs_win(plo, phi, base, c2, c3):
                    return x2t[plo:phi, base:base + c2 * 90].rearrange(
                        "p (a b c) -> p a b c", a=c2, b=9, c=D4P)[
                        :, :, 0:c3, 0:9]

                for r in range(R2 - 1, -1, -1):
                    # both o2s-subgroups batched per pass so the PE tile mode
                    # toggles once per direction, not per tap
                    groups = []
                    for (o2s, c2g) in ((0, 5), (5, 4)):
                        n = c2g * 81
                        ps2 = pp.tile([128, 512], F32, tag="ps")
                        groups.append((ps2[:, :n].rearrange(
                            "p (a b c) -> p a b c", a=c2g, b=9, c=9),
                            o2s, c2g))
                    # pass 1: 27 j4-fused K=128 taps (interior first)
                    for (ps2v, o2s, c2g) in groups:
                        for ti, (j1, j2, j3) in enumerate(TAPS):
                            out_ap, base0, c2, c3 = tap_geom(
                                ps2v, r, o2s, c2g, j1, j2, j3)
                            nc.tensor.matmul(
                                out_ap,
                                w2at[:, ti * 128:(ti + 1) * 128],
                                rhs_win(0, 128, base0, c2, c3),
                                start=(ti == 0), stop=False)
                    # pass 2 (j4=2 taps): for each (j1,j2), taps j3=0 and
                    # j3=1 run as one dense K=128 matmul against x2st over
                    # the intersection o3 in [1,9) (x2st upper partitions
                    # hold x2[+10] = the j3+1 window); the j3=1 tap's o3=0
                    # sliver and the j3=2 tap run K zero-padded to 128.
                    for (ps2v, o2s, c2g) in groups:
                        for q, (j1, j2) in enumerate(
                                itertools.product(range(3), range(3))):
                            blk = r + j1
                            lo2 = max(o2s, 1 - j2)
                            hi2 = min(o2s + c2g, 10 - j2)
                            c2 = hi2 - lo2
                            d2base = blk * BLK + (lo2 + j2 - 1) * 90
                            # pair (j3=0 rows 0-63, j3=1 rows 64-127)
                            bp_ = d2base + 0 * D4P + 2
                            rhsp = x2st[:, bp_:bp_ + c2 * 90].rearrange(
                                "p (a b c) -> p a b c", a=c2, b=9, c=D4P)[
                                :, :, 0:8, 0:9]
                            nc.tensor.matmul(
                                ps2v[:, lo2 - o2s:hi2 - o2s, 1:9, :],
                                w2pt[:, q * 128:(q + 1) * 128],
                                rhsp,
                                start=False, stop=False)
                            # sliver: tap (j1,j2,1) at o3=0
                            bs_ = d2base + 0 * D4P + 2
                            rhss = x2t[:, bs_:bs_ + c2 * 90].rearrange(
                                "p (a b c) -> p a b c", a=c2, b=9, c=D4P)[
                                :, :, 0:1, 0:9]
                            tb = TI[(j1, j2, 1)]
                            nc.tensor.matmul(
                                ps2v[:, lo2 - o2s:hi2 - o2s, 0:1, :],
                                w2bt[:, tb * 128:(tb + 1) * 128],
                                rhss,
                                start=False, stop=False)
                        # j3=2 leftovers: pair (j1,0,2)+(j1,1,2) against
                        # x2ut (upper partitions hold x2[+90] = the j2+1
                        # window), plus the j2=1 tap's o2=0 sliver and the
                        # j2=2 single
                        for j1 in range(3):
                            blk = r + j1
                            lo2 = max(o2s, 1)
                            hi2 = min(o2s + c2g, 9)
                            c2 = hi2 - lo2
                            bq_ = blk * BLK + (lo2 - 1) * 90 + D4P + 2
                            rhsq = x2ut[:, bq_:bq_ + c2 * 90].rearrange(
                                "p (a b c) -> p a b c", a=c2, b=9, c=D4P)[
                                :, :, 0:8, 0:9]
                            nc.tensor.matmul(
                                ps2v[:, lo2 - o2s:hi2 - o2s, 0:8, :],
                                w2qt[:, j1 * 128:(j1 + 1) * 128],
                                rhsq,
                                start=False, stop=False)
                            if o2s == 0:
                                # (j1,1,2) at o2=0 (outside the pair window)
                                bs2 = blk * BLK + 0 * 90 + D4P + 2
                                rhs2 = x2t[:, bs2:bs2 + 90].rearrange(
                                    "p (a b c) -> p a b c",
                                    a=1, b=9, c=D4P)[:, :, 0:8, 0:9]
                                t1 = TI[(j1, 1, 2)]
                                nc.tensor.matmul(
                                    ps2v[:, 0:1, 0:8, :],
                                    w2bt[:, t1 * 128:(t1 + 1) * 128],
                                    rhs2,
                                    start=False, stop=False)
                            # single: tap (j1,2,2), o2 in [o2s, min(.,8))
                            hi2c = min(o2s + c2g, 8)
                            c2c = hi2c - o2s
                            bc_ = blk * BLK + (o2s + 1) * 90 + D4P + 2
                            rhsc = x2t[:, bc_:bc_ + c2c * 90].rearrange(
                                "p (a b c) -> p a b c", a=c2c, b=9, c=D4P)[
                                :, :, 0:8, 0:9]
                            tcq = TI[(j1, 2, 2)]
                            nc.tensor.matmul(
                                ps2v[:, 0:c2c, 0:8, :],
                                w2bt[:, tcq * 128:(tcq + 1) * 128],
                                rhsc,
                                start=False, stop=(j1 == 2))
                    for gi, (ps2v, o2s, c2g) in enumerate(groups):
                        dst = x3v[:, r, o2s:o2s + c2g, :, :]
                        if gi == 0:
                            nc.scalar.activation(dst, ps2v[:, :, :, :],
                                                 Relu, bias=b2t[:, :])
                        else:
                            nc.vector.tensor_scalar(
                                dst, ps2v[:, :, :, :], b2t[:, :], 0.0,
                                mybir.AluOpType.add, mybir.AluOpType.max)

                if rep == 0:
                    # late weights: emitted after conv2 so the x2 replica
                    # copies outrank them on the DMA queue; they still land
                    # long before conv3 needs them.
                    nc.sync.dma_start(wpt[:, :], wp_d[:, :])
                    nc.sync.dma_start(fcwt[:, :], fcw_d[:, :])

                # ---------------- conv3 + conv4 (1x1), chunk-interleaved ----
                Amax = mybir.AluOpType.max
                Aadd = mybir.AluOpType.add

                def evac(dst, src, bias, mh):
                    # PSUM evacuation alternates engines: ScalarE handles
                    # mh=0, VectorE mh=1, so neither engine serializes PE.
                    if mh == 0:
                        nc.scalar.activation(dst, src, Relu, bias=bias)
                    else:
                        nc.vector.tensor_scalar(dst, src, bias, 0.0,
                                                Aadd, Amax)

                def conv3_chunk(pos, sz):
                    for mh in range(2):
                        ps3 = pp2.tile([128, 512], F32, tag="ps2")
                        nc.tensor.matmul(
                            ps3[:, :sz],
                            w3t[:, mh * 128:(mh + 1) * 128],
                            x3t[:, pos:pos + sz],
                            start=True, stop=True)
                        evac(x4t[:, mh * N3 + pos:mh * N3 + pos + sz],
                             ps3[:, :sz], b3t[:, mh:mh + 1], mh)

                def conv4_chunk(pos, sz):
                    for mh in range(2):
                        ps4 = pp2.tile([128, 512], F32, tag="ps2")
                        nc.tensor.matmul(
                            ps4[:, :sz],
                            w4t[:, (mh * 2) * 128:(mh * 2 + 1) * 128],
                            x4t[:, pos:pos + sz],
                            start=True, stop=False)
                        nc.tensor.matmul(
                            ps4[:, :sz],
                            w4t[:, (mh * 2 + 1) * 128:(mh * 2 + 2) * 128],
                            x4t[:, N3 + pos:N3 + pos + sz],
                            start=False, stop=True)
                        evac(x4bt[:, mh * N3 + pos:mh * N3 + pos + sz],
                             ps4[:, :sz], b4t[:, mh:mh + 1], mh)

                # two-chunk software pipeline: conv4(k) trails conv3(k+2)
                rchunks = list(reversed(chunks))
                conv3_chunk(*rchunks[0])
                conv3_chunk(*rchunks[1])
                for i in range(len(rchunks)):
                    if i + 2 < len(rchunks):
                        conv3_chunk(*rchunks[i + 2])
                    conv4_chunk(*rchunks[i])

                # ---------------- conv5 (1x1, s=2, 128c->64c) ----------------
                for rr in range(R5 - 1, -1, -1):
                    ps5 = pp2.tile([128, 512], F32, tag="ps2")
                    for mb in range(2):
                        rhs = x4bv[:, mb, 2 * rr, 0:9:2, 0:9:2, 0:9:2]
                        nc.tensor.matmul(
                            ps5[:, :125],
                            w5t[:, mb * 128:(mb + 1) * 128],
                            rhs,
                            start=(mb == 0), stop=(mb == 1))
                    nc.scalar.activation(
                        x5t[:, rr * 125:(rr + 1) * 125],
                        ps5[:, :125],
                        Relu, bias=b5t[:, :])
                    nc.vector.tensor_mul(
                        prodt[:, rr * 125:(rr + 1) * 125],
                        x5t[:, rr * 125:(rr + 1) * 125],
                        fcwt[:, rr * 125:(rr + 1) * 125])
                    nc.vector.reduce_sum(
                        fct[:, rr:rr + 1],
                        prodt[:, rr * 125:(rr + 1) * 125],
                        axis=mybir.AxisListType.X)


            nc.sync.dma_start(out_d[:, :], fct[:, :])

    nc.compile()
    return nc


# ---------------- host-side data prep ----------------

def _prep_weights(inputs):
    f32 = np.float32
    w1r = np.asarray(inputs["w1r"], f32)[:, 0]   # [32, 3,3,3,3]
    w1i = np.asarray(inputs["w1i"], f32)[:, 0]
    # [t27, j4, co]
    w1r_t = w1r.transpose(1, 2, 3, 4, 0).reshape(27, 3, 32)
    w1i_t = w1i.transpose(1, 2, 3, 4, 0).reshape(27, 3, 32)
    # two variants: cols [0:384) contract slab rows 0-3 (K rows 0-54),
    # cols [384:768) contract slab rows 4-6 (K rows 64-118)
    W1 = np.zeros((128, 6 * 128), f32)
    for v in range(2):
        k0 = 64 * v
        for j4 in range(3):
            c0 = (v * 3 + j4) * 128
            W1[k0 + 0:k0 + 27, c0:c0 + 32] = w1r_t[:, j4]
            W1[k0 + 0:k0 + 27, c0 + 32:c0 + 64] = w1i_t[:, j4]
            W1[k0 + 27:k0 + 54, c0:c0 + 32] = -w1i_t[:, j4]
            W1[k0 + 27:k0 + 54, c0 + 32:c0 + 64] = w1r_t[:, j4]
        W1[k0 + 54, v * 3 * 128:v * 3 * 128 + 32] = np.asarray(inputs["b1r"], f32)
        W1[k0 + 54, v * 3 * 128 + 32:v * 3 * 128 + 64] = np.asarray(inputs["b1i"], f32)

    w2r = np.asarray(inputs["w2r"], f32)   # [64, 32, 3,3,3,3]
    w2i = np.asarray(inputs["w2i"], f32)
    # [t27, j4, ci, co]
    w2r_t = w2r.transpose(2, 3, 4, 5, 1, 0).reshape(27, 3, 32, 64)
    w2i_t = w2i.transpose(2, 3, 4, 5, 1, 0).reshape(27, 3, 32, 64)
    W2a = np.zeros((128, 27 * 128), f32)
    W2b = np.zeros((64, 27 * 128), f32)
    # columns packed in TAPS (kernel emission) order
    for ti, (j1, j2, j3) in enumerate(TAPS):
        t = j1 * 9 + j2 * 3 + j3
        for jj, r0 in ((0, 0), (1, 64)):
            W2a[r0 + 0:r0 + 32, ti * 128:ti * 128 + 64] = w2r_t[t, jj]
            W2a[r0 + 0:r0 + 32, ti * 128 + 64:(ti + 1) * 128] = w2i_t[t, jj]
            W2a[r0 + 32:r0 + 64, ti * 128:ti * 128 + 64] = -w2i_t[t, jj]
            W2a[r0 + 32:r0 + 64, ti * 128 + 64:(ti + 1) * 128] = w2r_t[t, jj]
        W2b[0:32, ti * 128:ti * 128 + 64] = w2r_t[t, 2]
        W2b[0:32, ti * 128 + 64:(ti + 1) * 128] = w2i_t[t, 2]
        W2b[32:64, ti * 128:ti * 128 + 64] = -w2i_t[t, 2]
        W2b[32:64, ti * 128 + 64:(ti + 1) * 128] = w2r_t[t, 2]
    # j3-pair weights for the j4=2 pass: block q=(j1,j2) holds tap
    # (j1,j2,0) on K rows 0-63 and tap (j1,j2,1) on rows 64-127
    W2p = np.zeros((128, 9 * 128), f32)
    for q, (j1, j2) in enumerate(itertools.product(range(3), range(3))):
        for j3, k0 in ((0, 0), (1, 64)):
            t = j1 * 9 + j2 * 3 + j3
            W2p[k0 + 0:k0 + 32, q * 128:q * 128 + 64] = w2r_t[t, 2]
            W2p[k0 + 0:k0 + 32, q * 128 + 64:(q + 1) * 128] = w2i_t[t, 2]
            W2p[k0 + 32:k0 + 64, q * 128:q * 128 + 64] = -w2i_t[t, 2]
            W2p[k0 + 32:k0 + 64, q * 128 + 64:(q + 1) * 128] = w2r_t[t, 2]
    # j2-pair weights for the j3=2,j4=2 leftovers: block j1 holds tap
    # (j1,0,2) on K rows 0-63 and (j1,1,2) on rows 64-127
    W2q = np.zeros((128, 3 * 128), f32)
    for j1 in range(3):
        for j2, k0 in ((0, 0), (1, 64)):
            t = j1 * 9 + j2 * 3 + 2
            W2q[k0 + 0:k0 + 32, j1 * 128:j1 * 128 + 64] = w2r_t[t, 2]
            W2q[k0 + 0:k0 + 32, j1 * 128 + 64:(j1 + 1) * 128] = w2i_t[t, 2]
            W2q[k0 + 32:k0 + 64, j1 * 128:j1 * 128 + 64] = -w2i_t[t, 2]
            W2q[k0 + 32:k0 + 64, j1 * 128 + 64:(j1 + 1) * 128] = w2r_t[t, 2]
    B2 = np.concatenate([np.asarray(inputs["b2r"], f32),
                         np.asarray(inputs["b2i"], f32)])[:, None]

    w3r = np.asarray(inputs["w3r"], f32).reshape(128, 64)
    w3i = np.asarray(inputs["w3i"], f32).reshape(128, 64)
    W3 = np.zeros((128, 2 * 128), f32)
    W3[0:64, 0:128] = w3r.T
    W3[64:128, 0:128] = -w3i.T
    W3[0:64, 128:256] = w3i.T
    W3[64:128, 128:256] = w3r.T
    B3 = np.stack([np.asarray(inputs["b3r"], f32),
                   np.asarray(inputs["b3i"], f32)], axis=1)

    w4r = np.asarray(inputs["w4r"], f32).reshape(128, 128)
    w4i = np.asarray(inputs["w4i"], f32).reshape(128, 128)
    W4 = np.zeros((128, 4 * 128), f32)
    W4[:, 0:128] = w4r.T
    W4[:, 128:256] = -w4i.T
    W4[:, 256:384] = w4i.T
    W4[:, 384:512] = w4r.T
    B4 = np.stack([np.asarray(inputs["b4r"], f32),
                   np.asarray(inputs["b4i"], f32)], axis=1)

    w5r = np.asarray(inputs["w5r"], f32).reshape(64, 128)
    w5i = np.asarray(inputs["w5i"], f32).reshape(64, 128)
    W5 = np.zeros((128, 2 * 128), f32)
    W5[:, 0:64] = w5r.T
    W5[:, 64:128] = w5i.T
    W5[:, 128:192] = -w5i.T
    W5[:, 192:256] = w5r.T
    B5 = np.concatenate([np.asarray(inputs["b5r"], f32),
                         np.asarray(inputs["b5i"], f32)])[:, None]

    BP = np.zeros((128, 6), f32)
    BP[:, 0:1] = B2
    BP[:, 1:3] = B3
    BP[:, 3:5] = B4
    BP[:, 5:6] = B5
    WP = np.zeros((128, 8 * 128), f32)
    WP[:, 0:256] = W3
    WP[:, 256:768] = W4
    WP[:, 768:1024] = W5
    return {
        "w1": W1.astype(BF), "w2a": W2a.astype(BF), "w2b": W2b.astype(BF),
        "w2p": W2p.astype(BF), "w2q": W2q.astype(BF),
        "bp": BP, "wp": WP.astype(BF),
    }


def _prep_x1(xr_b, xi_b, h):
    """Conv1 input slab for one (batch, half), packed [128, 4*CHK1] bf16:
    partition p < 64 holds slab row r=chunk of tap p; partition 64+p holds
    row chunk+4 (row 7 slot is zero)."""
    S = np.zeros((64, R1 + 1, 9, 9, D4S), np.float32)
    glo = max(0, 4 * h - 1)
    ghi = min(8, 4 * h + 5)
    rlo = glo - (4 * h - 1)
    rhi = ghi - (4 * h - 1) + 1
    for t, (j1, j2, j3) in enumerate(itertools.product(range(3), repeat=3)):
        subr = xr_b[j1:j1 + 17:2, j2:j2 + 17:2, j3:j3 + 17:2, :D4S]
        subi = xi_b[j1:j1 + 17:2, j2:j2 + 17:2, j3:j3 + 17:2, :D4S]
        S[t, rlo:rhi] = subr[glo:ghi + 1]
        S[27 + t, rlo:rhi] = subi[glo:ghi + 1]
    S[54, rlo:rhi] = 1.0
    S = S.reshape(64, 2, 4 * CHK1)
    return np.concatenate([S[:, 0], S[:, 1]], axis=0).astype(BF)


def _prep_fcw(fcw, h):
    out = np.zeros((128, N5), np.float32)
    f = np.asarray(fcw, np.float32).reshape(-1)
    for rr in range(R5):
        g5 = rr + 2 * h
        if h == 1 and rr == 0:
            continue  # overlap row masked on half 1
        out[:, rr * 125:(rr + 1) * 125] = f[g5 * 125:(g5 + 1) * 125][None, :]
    return out


def _make_in_maps(inputs):
    wkey = id(inputs.get("w1r"))
    if _CACHE.get("wkey") != wkey:
        _CACHE["wmaps"] = _prep_weights(inputs)
        _CACHE["wkey"] = wkey
    wmaps = _CACHE["wmaps"]
    xr = np.asarray(inputs["xr"], np.float32)
    xi = np.asarray(inputs["xi"], np.float32)
    fcw = inputs["fcw"]
    in_maps = []
    for core in range(8):
        b, h = core // 2, core % 2
        m = dict(wmaps)
        m["x1"] = _prep_x1(xr[b, 0], xi[b, 0], h)
        m["fcw"] = _prep_fcw(fcw, h)
        in_maps.append(m)
    return in_maps


def kernel(**inputs):
    if "nc" not in _CACHE:
        _CACHE["nc"] = _build_nc()
    nc = _CACHE["nc"]

    in_maps = _make_in_maps(inputs)
    res = run_bass_kernel_spmd(nc, in_maps, core_ids=list(range(8)))

    fcb = np.asarray(inputs["fcb"], np.float32)
    yr = np.zeros((NB, 64, 1), np.float32)
    yi = np.zeros((NB, 64, 1), np.float32)
    for b in range(NB):
        p0 = res.results[2 * b]["out"]
        p1 = res.results[2 * b + 1]["out"]
        s = (p0 + p1).sum(axis=1, keepdims=True)
        yr[b] = s[0:64] + fcb[0]
        yi[b] = s[64:128]
    return np.stack([yr, yi]).astype(np.float32)
